# revision 1
# baseline (speedup 1.0000x reference)
"""BoneMeshGAT Trainium2 kernel: 3-layer GAT + BN/ELU on 8 NeuronCores.

Sharding: edges partitioned by dst across 8 cores (each core owns a
contiguous 12.5k-node dst shard and all edges into it). Per layer, each
core aggregates its shard's nodes (segment-softmax attention via
selection-matrix matmuls into PSUM), BN stats are AllReduced, the next
layer's gather table is rebuilt per-shard and AllGathered. Gathers of
per-edge source rows use indirect DMA. The program is JIT-built per
input graph (block/tile structure baked from the sorted edge list).
"""
import sys
sys.path.insert(0, "/opt/trn_rl_repo")
import numpy as np

import concourse.bass as bass
import concourse.mybir as mybir
import concourse.tile as tile
from concourse import bacc, bass_utils

N, E, G = 100000, 1000000, 16
H, D, IN = 4, 64, 4
HID = H * D
NC_ = 8
P = 128
NSH = N // NC_              # 12500 nodes per shard
NBLK = (NSH + P - 1) // P   # 98 blocks
NSHP = NBLK * P             # 12544 padded rows
EPS = 1e-5
F32 = mybir.dt.float32
I32 = mybir.dt.int32


def _fold_attn(W, a_src, a_dst, heads, d):
    # asrc_vec[i, h] = sum_d W[i, h*d+dd] * a_src[h, dd]
    Wr = W.reshape(W.shape[0], heads, d)
    av = np.einsum('ihd,hd->ih', Wr, a_src).astype(np.float32)
    bv = np.einsum('ihd,hd->ih', Wr, a_dst).astype(np.float32)
    return av, bv


def _host_prep(x, edge_index):
    src = edge_index[0].astype(np.int64)
    dst = edge_index[1].astype(np.int64)
    order = np.argsort(dst, kind='stable')
    s_s, d_s = src[order], dst[order]
    core_bounds = np.searchsorted(d_s, np.arange(0, N + 1, NSH))
    cnts = np.zeros((NC_, NBLK), np.int64)
    per_core = []
    for c in range(NC_):
        e0, e1 = core_bounds[c], core_bounds[c + 1]
        sc, dc = s_s[e0:e1], d_s[e0:e1] - c * NSH
        db = dc // P
        cnts[c] = np.bincount(db, minlength=NBLK)
        per_core.append((sc, dc, db))
    tpb = np.maximum(1, -(-cnts.max(axis=0) // P))  # tiles per block, shared
    TT = int(tpb.sum())
    tile_start = np.zeros(NBLK + 1, np.int64)
    tile_start[1:] = np.cumsum(tpb)

    src_slab = np.zeros((NC_, P, TT), np.int32)
    dcol_slab = np.full((NC_, P, TT), 999.0, np.float32)
    xsrcT_slab = np.zeros((NC_, IN, TT * P), np.float32)
    for c in range(NC_):
        sc, dc, db = per_core[c]
        blk_off = np.zeros(NBLK + 1, np.int64)
        blk_off[1:] = np.cumsum(cnts[c])
        for j in range(NBLK):
            e0, e1 = blk_off[j], blk_off[j + 1]
            n = e1 - e0
            t0 = tile_start[j]
            for k in range(int(tpb[j])):
                a = e0 + k * P
                b = min(a + P, e1)
                if b <= a:
                    break
                m = b - a
                col = t0 + k
                srcs = sc[a:b]
                # remap to padded AllGather row ids
                rows = (srcs // NSH) * NSHP + (srcs % NSH)
                src_slab[c, :m, col] = rows.astype(np.int32)
                dcol_slab[c, :m, col] = (dc[a:b] - j * P).astype(np.float32)
                xsrcT_slab[c, :, col * P:col * P + m] = x[srcs].T
    return tpb, TT, src_slab, dcol_slab, xsrcT_slab


def _build_program(TT, tpb, weights):
    (M1, W2f, W3f, g1, b1, g2, b2, g3, b3) = weights
    RW2, RW3 = 264, 66
    nc = bacc.Bacc("TRN2", target_bir_lowering=False, debug=False,
                   num_devices=NC_)

    def din(name, shape, dt=F32):
        return nc.dram_tensor(name, shape, dt, kind="ExternalInput").ap()

    i_src = din("i_src", [P, TT], I32)
    i_dcol = din("i_dcol", [P, TT])
    i_xsrcT = din("i_xsrcT", [IN, TT * P])
    i_xshT = din("i_xshT", [IN, NSHP])
    i_iota = din("i_iota", [P, P])
    i_ident = din("i_ident", [P, P])
    i_ones1 = din("i_ones1", [1, P])
    i_M1 = din("i_M1", [IN, RW2])
    i_W2f = din("i_W2f", [HID, RW2])
    i_W3f = din("i_W3f", [HID, RW3])
    i_g1 = din("i_g1", [1, HID]); i_b1 = din("i_b1", [1, HID])
    i_g2 = din("i_g2", [1, HID]); i_b2 = din("i_b2", [1, HID])
    i_g3 = din("i_g3", [1, D]);   i_b3 = din("i_b3", [1, D])
    o_h3 = nc.dram_tensor("o_h3", [NSHP, D], F32, kind="ExternalOutput").ap()

    with tile.TileContext(nc) as tc:
        with (
            tc.tile_pool(name="dram", bufs=1, space="DRAM") as dp,
            tc.tile_pool(name="const", bufs=1) as cp,
        ):
            agg1 = dp.tile([NSHP, HID], F32)
            agg2 = dp.tile([NSHP, HID], F32)
            agg3 = dp.tile([NSHP, D], F32)
            t2_own = dp.tile([NSHP, RW2], F32)
            t2_full = dp.tile([NC_ * NSHP, RW2], F32, addr_space="Shared")
            t3_own = dp.tile([NSHP, RW3], F32)
            t3_full = dp.tile([NC_ * NSHP, RW3], F32, addr_space="Shared")
            st1_in = dp.tile([2, HID], F32)
            st1_out = dp.tile([2, HID], F32, addr_space="Shared")
            st2_in = dp.tile([2, HID], F32)
            st2_out = dp.tile([2, HID], F32, addr_space="Shared")
            st3_in = dp.tile([2, D], F32)
            st3_out = dp.tile([2, D], F32, addr_space="Shared")

            iota = cp.tile([P, P], F32)
            nc.sync.dma_start(iota[:], i_iota[:])
            ident = cp.tile([P, P], F32)
            nc.sync.dma_start(ident[:], i_ident[:])
            ones1 = cp.tile([1, P], F32)
            nc.sync.dma_start(ones1[:], i_ones1[:])
            onesc = cp.tile([P, 1], F32)
            nc.vector.memset(onesc[:], 1.0)
            src_slab = cp.tile([P, TT], I32)
            nc.sync.dma_start(src_slab[:], i_src[:])
            dcol_slab = cp.tile([P, TT], F32)
            nc.sync.dma_start(dcol_slab[:], i_dcol[:])
            M1sb = cp.tile([IN, RW2], F32)
            nc.sync.dma_start(M1sb[:], i_M1[:])

            # ---- L1 prep: s_dst1 per block from x_shardT @ M1[:,260:264]
            sdst1 = cp.tile([P, 4 * NBLK], F32)
            with tc.tile_pool(name="pp0", bufs=2, space="PSUM") as pp0, \
                 tc.tile_pool(name="sp0", bufs=2) as sp0:
                xsh = sp0.tile([IN, NSHP], F32, tag="xsh")
                nc.sync.dma_start(xsh[:], i_xshT[:])
                for j in range(NBLK):
                    ps = pp0.tile([P, 4], F32, tag="ps")
                    nc.tensor.matmul(out=ps[:], lhsT=xsh[:, j * P:(j + 1) * P],
                                     rhs=M1sb[:, 260:264], start=True, stop=True)
                    nc.vector.tensor_copy(sdst1[:, 4 * j:4 * j + 4], ps[:])

            CHT = 16  # xsrcT tiles per SBUF chunk (L1)

            def edge_phase(layer, table_ap, agg_ap, sdst_own_ap, sdst_cols,
                           xsrcT_ap, Hn, RW, numw, st_in_ap):
                """One GAT layer edge aggregation into agg_ap [NSHP, numw],
                with BN-stat partial sums fused in (written to st_in_ap)."""
                rhw = numw + Hn   # rhs/acc width
                ps_bufs = 1 if layer == 1 else 2
                with (
                    tc.tile_pool(name=f"g{layer}", bufs=8) as gp,
                    tc.tile_pool(name=f"w{layer}", bufs=4) as wp,
                    tc.tile_pool(name=f"b{layer}", bufs=3) as bp,
                    tc.tile_pool(name=f"x{layer}", bufs=2) as xp,
                    tc.tile_pool(name=f"ps{layer}", bufs=ps_bufs, space="PSUM") as pp,
                    tc.tile_pool(name=f"pa{layer}", bufs=2, space="PSUM") as pa,
                    tc.tile_pool(name=f"pq{layer}", bufs=1, space="PSUM") as pq,
                ):
                    s0 = pq.tile([1, numw], F32, tag="s0")
                    s1 = pq.tile([1, numw], F32, tag="s1")
                    xchunk = None
                    t = 0
                    for j in range(NBLK):
                        if layer == 1:
                            sdb = sdst1
                            sdb_sl = (slice(None), slice(4 * j, 4 * j + Hn))
                        else:
                            sdb = bp.tile([P, Hn], F32, tag="sdb")
                            nc.sync.dma_start(
                                sdb[:], sdst_own_ap[j * P:(j + 1) * P,
                                                    sdst_cols[0]:sdst_cols[1]])
                            sdb_sl = (slice(None), slice(None))
                        acc = pa.tile([P, rhw], F32, tag="acc")
                        ntile = int(tpb[j])
                        for k in range(ntile):
                            if layer == 1:
                                if t % CHT == 0:
                                    xchunk = xp.tile([IN, CHT * P], F32,
                                                     tag="xchunk")
                                    ncols = min(CHT * P, TT * P - t * P)
                                    nc.sync.dma_start(
                                        xchunk[:, :ncols],
                                        xsrcT_ap[:, t * P:t * P + ncols])
                                co = (t % CHT) * P
                                gps = pp.tile([P, RW], F32, tag="gps")
                                nc.tensor.matmul(
                                    out=gps[:],
                                    lhsT=xchunk[:, co:co + P],
                                    rhs=M1sb[:], start=True, stop=True)
                                g = wp.tile([P, RW], F32, tag="gsb")
                                nc.vector.tensor_copy(g[:], gps[:])
                            else:
                                g = gp.tile([P, RW], F32, tag="gsb")
                                nc.gpsimd.indirect_dma_start(
                                    out=g[:], out_offset=None, in_=table_ap,
                                    in_offset=bass.IndirectOffsetOnAxis(
                                        ap=src_slab[:, t:t + 1], axis=0))
                            sel = wp.tile([P, P], F32, tag="sel")
                            nc.vector.tensor_tensor(
                                out=sel[:],
                                in0=dcol_slab[:, t:t + 1].to_broadcast([P, P]),
                                in1=iota[:], op=mybir.AluOpType.is_equal)
                            selT_ps = pp.tile([P, P], F32, tag="selTps")
                            nc.tensor.transpose(out=selT_ps[:], in_=sel[:],
                                                identity=ident[:])
                            selT = wp.tile([P, P], F32, tag="selT")
                            nc.vector.tensor_copy(selT[:], selT_ps[:])
                            sd_ps = pp.tile([P, Hn], F32, tag="sdps")
                            nc.tensor.matmul(out=sd_ps[:], lhsT=selT[:],
                                             rhs=sdb[sdb_sl],
                                             start=True, stop=True)
                            lg = wp.tile([P, Hn], F32, tag="lg")
                            nc.vector.tensor_add(lg[:], g[:, numw:numw + Hn],
                                                 sd_ps[:])
                            lk = wp.tile([P, Hn], F32, tag="lk")
                            nc.vector.scalar_tensor_tensor(
                                out=lk[:], in0=lg[:], scalar=0.2, in1=lg[:],
                                op0=mybir.AluOpType.mult,
                                op1=mybir.AluOpType.max)
                            rhs = wp.tile([P, rhw], F32, tag="rhs")
                            nc.scalar.activation(
                                rhs[:, numw:numw + Hn], lk[:],
                                mybir.ActivationFunctionType.Exp)
                            for h in range(Hn):
                                nc.vector.tensor_scalar(
                                    out=rhs[:, h * D:(h + 1) * D],
                                    in0=g[:, h * D:(h + 1) * D],
                                    scalar1=rhs[:, numw + h:numw + h + 1],
                                    scalar2=None,
                                    op0=mybir.AluOpType.mult)
                            nc.tensor.matmul(out=acc[:], lhsT=sel[:], rhs=rhs[:],
                                             start=(k == 0), stop=(k == ntile - 1))
                            t += 1
                        # softmax division: num/(den+1e-8)
                        dp_ = bp.tile([P, Hn], F32, tag="dp")
                        nc.vector.tensor_scalar_add(dp_[:], acc[:, numw:numw + Hn],
                                                    1e-8)
                        rec = bp.tile([P, Hn], F32, tag="rec")
                        nc.vector.reciprocal(rec[:], dp_[:])
                        aggb = bp.tile([P, numw], F32, tag="aggb")
                        for h in range(Hn):
                            nc.vector.tensor_scalar(
                                out=aggb[:, h * D:(h + 1) * D],
                                in0=acc[:, h * D:(h + 1) * D],
                                scalar1=rec[:, h:h + 1], scalar2=None,
                                op0=mybir.AluOpType.mult)
                        nc.sync.dma_start(agg_ap[j * P:(j + 1) * P, :], aggb[:])
                        sq = bp.tile([P, numw], F32, tag="sq")
                        nc.vector.tensor_mul(sq[:], aggb[:], aggb[:])
                        nc.tensor.matmul(out=s0[:], lhsT=onesc[:], rhs=aggb[:],
                                         start=(j == 0), stop=(j == NBLK - 1))
                        nc.tensor.matmul(out=s1[:], lhsT=onesc[:], rhs=sq[:],
                                         start=(j == 0), stop=(j == NBLK - 1))
                    assert t == TT
                    c0 = bp.tile([1, numw], F32, tag="c0")
                    nc.vector.tensor_copy(c0[:], s0[:])
                    c1 = bp.tile([1, numw], F32, tag="c1")
                    nc.vector.tensor_copy(c1[:], s1[:])
                    nc.sync.dma_start(st_in_ap[0:1, :], c0[:])
                    nc.sync.dma_start(st_in_ap[1:2, :], c1[:])

            def stats_ar(st_in_ap, st_out_ap):
                nc.gpsimd.collective_compute(
                    "AllReduce", mybir.AluOpType.add,
                    ins=[st_in_ap], outs=[st_out_ap],
                    replica_groups=[list(range(NC_))])

            def bn_params(st_out_ap, CH, g_ap, b_ap, pool, tag=""):
                """Returns replicated A,B tiles [P, CH]: x_hat*g+b = x*A+B."""
                st0 = pool.tile([1, CH], F32, tag="stl0")
                nc.sync.dma_start(st0[:], st_out_ap[0:1, :])
                st1_ = pool.tile([1, CH], F32, tag="stl1")
                nc.sync.dma_start(st1_[:], st_out_ap[1:2, :])
                grow = pool.tile([1, CH], F32, tag="grow")
                nc.sync.dma_start(grow[:], g_ap[:])
                brow = pool.tile([1, CH], F32, tag="brow")
                nc.sync.dma_start(brow[:], b_ap[:])
                mu = pool.tile([1, CH], F32, tag="mu")
                nc.vector.tensor_scalar_mul(mu[:], st0[:], 1.0 / N)
                msq = pool.tile([1, CH], F32, tag="msq")
                nc.vector.tensor_scalar_mul(msq[:], st1_[:], 1.0 / N)
                var = pool.tile([1, CH], F32, tag="var")
                nc.vector.tensor_mul(var[:], mu[:], mu[:])
                nc.vector.tensor_sub(var[:], msq[:], var[:])
                nc.vector.tensor_scalar_add(var[:], var[:], EPS)
                sd = pool.tile([1, CH], F32, tag="sd")
                nc.scalar.activation(sd[:], var[:],
                                     mybir.ActivationFunctionType.Sqrt)
                r = pool.tile([1, CH], F32, tag="r")
                nc.vector.reciprocal(r[:], sd[:])
                arow = pool.tile([1, CH], F32, tag="arow")
                nc.vector.tensor_mul(arow[:], r[:], grow[:])
                brow2 = pool.tile([1, CH], F32, tag="brow2")
                nc.vector.tensor_mul(brow2[:], mu[:], arow[:])
                nc.vector.tensor_sub(brow2[:], brow[:], brow2[:])
                with tc.tile_pool(name=f"pbn{tag}", bufs=1, space="PSUM") as pb:
                    aps = pb.tile([P, CH], F32, tag="aps")
                    nc.tensor.matmul(out=aps[:], lhsT=ones1[:], rhs=arow[:],
                                     start=True, stop=True)
                    A = pool.tile([P, CH], F32, tag="Arep")
                    nc.vector.tensor_copy(A[:], aps[:])
                    bps = pb.tile([P, CH], F32, tag="bps")
                    nc.tensor.matmul(out=bps[:], lhsT=ones1[:], rhs=brow2[:],
                                     start=True, stop=True)
                    B = pool.tile([P, CH], F32, tag="Brep")
                    nc.vector.tensor_copy(B[:], bps[:])
                return A, B

            def bn_elu_block(xb, A, B, CH, pool):
                """BN apply + ELU in place; returns act tile [P, CH]."""
                z = pool.tile([P, CH], F32, tag="z")
                nc.vector.tensor_mul(z[:], xb[:], A[:])
                nc.vector.tensor_add(z[:], z[:], B[:])
                m0 = pool.tile([P, CH], F32, tag="m0")
                nc.vector.tensor_scalar_min(m0[:], z[:], 0.0)
                e = pool.tile([P, CH], F32, tag="e")
                nc.scalar.activation(e[:], m0[:],
                                     mybir.ActivationFunctionType.Exp)
                p0 = pool.tile([P, CH], F32, tag="p0")
                nc.vector.tensor_scalar_max(p0[:], z[:], 0.0)
                act = pool.tile([P, CH], F32, tag="act")
                nc.vector.tensor_add(act[:], p0[:], e[:])
                nc.vector.tensor_scalar_sub(act[:], act[:], 1.0)
                return act

            def build_pass(agg_ap, st_out_ap, g_ap, b_ap, Wf_ap, RW,
                           t_own_ap, t_full_ap, tag=""):
                CH = HID
                with (
                    tc.tile_pool(name=f"bps{tag}", bufs=3) as sp,
                    tc.tile_pool(name=f"bpc{tag}", bufs=1) as cpool,
                    tc.tile_pool(name=f"bpp{tag}", bufs=2, space="PSUM") as pp,
                    tc.tile_pool(name=f"bpt{tag}", bufs=2, space="PSUM") as pt,
                ):
                    A, B = bn_params(st_out_ap, CH, g_ap, b_ap, cpool, tag)
                    Wsb0 = cpool.tile([P, RW], F32, tag="Wsb0")
                    nc.sync.dma_start(Wsb0[:], Wf_ap[0:P, :])
                    Wsb1 = cpool.tile([P, RW], F32, tag="Wsb1")
                    nc.sync.dma_start(Wsb1[:], Wf_ap[P:2 * P, :])
                    Wsb = [Wsb0, Wsb1]
                    for j in range(NBLK):
                        xb = sp.tile([P, CH], F32, tag="xb")
                        nc.sync.dma_start(xb[:], agg_ap[j * P:(j + 1) * P, :])
                        act = bn_elu_block(xb, A, B, CH, sp)
                        tp = pt.tile([P, RW], F32, tag="tp")
                        for k in range(2):
                            trp = pp.tile([P, P], F32, tag="trp")
                            nc.tensor.transpose(
                                out=trp[:], in_=act[:, k * P:(k + 1) * P],
                                identity=ident[:])
                            atk = sp.tile([P, P], F32, tag="atk")
                            nc.vector.tensor_copy(atk[:], trp[:])
                            nc.tensor.matmul(out=tp[:], lhsT=atk[:],
                                             rhs=Wsb[k][:],
                                             start=(k == 0), stop=(k == 1))
                        tt = sp.tile([P, RW], F32, tag="tt")
                        nc.vector.tensor_copy(tt[:], tp[:])
                        nc.sync.dma_start(t_own_ap[j * P:(j + 1) * P, :], tt[:])
                nc.gpsimd.collective_compute(
                    "AllGather", mybir.AluOpType.bypass,
                    ins=[t_own_ap], outs=[t_full_ap],
                    replica_groups=[list(range(NC_))])

            # ================= pipeline =================
            edge_phase(1, None, agg1[:], None, None,
                       i_xsrcT[:], H, RW2, HID, st1_in[:])
            stats_ar(st1_in[:], st1_out[:])
            build_pass(agg1[:], st1_out[:], i_g1[:], i_b1[:], i_W2f[:], RW2,
                       t2_own[:], t2_full[:], tag="1")
            edge_phase(2, t2_full[:], agg2[:], t2_own[:], (260, 264),
                       None, H, RW2, HID, st2_in[:])
            stats_ar(st2_in[:], st2_out[:])
            build_pass(agg2[:], st2_out[:], i_g2[:], i_b2[:], i_W3f[:], RW3,
                       t3_own[:], t3_full[:], tag="2")
            edge_phase(3, t3_full[:], agg3[:], t3_own[:], (65, 66),
                       None, 1, RW3, D, st3_in[:])
            stats_ar(st3_in[:], st3_out[:])
            # final BN3 + ELU -> h3 output
            with (
                tc.tile_pool(name="f3", bufs=3) as sp,
                tc.tile_pool(name="f3c", bufs=1) as cpool,
            ):
                A3, B3 = bn_params(st3_out[:], D, i_g3[:], i_b3[:], cpool, "3")
                for j in range(NBLK):
                    xb = sp.tile([P, D], F32, tag="xb")
                    nc.sync.dma_start(xb[:], agg3[j * P:(j + 1) * P, :])
                    act = bn_elu_block(xb, A3, B3, D, sp)
                    nc.sync.dma_start(o_h3[j * P:(j + 1) * P, :], act[:])
    nc.compile()
    return nc


def kernel(**inputs):
    x = np.asarray(inputs["x"], np.float32)
    edge_index = np.asarray(inputs["edge_index"], np.int32)
    batch = np.asarray(inputs["batch"], np.int64)
    W1 = np.asarray(inputs["W1"], np.float32)
    a_src1 = np.asarray(inputs["a_src1"], np.float32)
    a_dst1 = np.asarray(inputs["a_dst1"], np.float32)
    g1 = np.asarray(inputs["g1"], np.float32); b1 = np.asarray(inputs["b1"], np.float32)
    W2 = np.asarray(inputs["W2"], np.float32)
    a_src2 = np.asarray(inputs["a_src2"], np.float32)
    a_dst2 = np.asarray(inputs["a_dst2"], np.float32)
    g2 = np.asarray(inputs["g2"], np.float32); b2 = np.asarray(inputs["b2"], np.float32)
    W3 = np.asarray(inputs["W3"], np.float32)
    a_src3 = np.asarray(inputs["a_src3"], np.float32)
    a_dst3 = np.asarray(inputs["a_dst3"], np.float32)
    g3 = np.asarray(inputs["g3"], np.float32); b3 = np.asarray(inputs["b3"], np.float32)
    Wc1 = np.asarray(inputs["Wc1"], np.float32); bc1 = np.asarray(inputs["bc1"], np.float32)
    gc1 = np.asarray(inputs["gc1"], np.float32); bnc1 = np.asarray(inputs["bnc1"], np.float32)
    Wc2 = np.asarray(inputs["Wc2"], np.float32); bc2 = np.asarray(inputs["bc2"], np.float32)
    Wc3 = np.asarray(inputs["Wc3"], np.float32); bc3 = np.asarray(inputs["bc3"], np.float32)

    tpb, TT, src_slab, dcol_slab, xsrcT_slab = _host_prep(x, edge_index)

    av1, bv1 = _fold_attn(W1, a_src1, a_dst1, H, D)
    M1 = np.concatenate([W1, av1, bv1], axis=1).astype(np.float32)      # [4,264]
    av2, bv2 = _fold_attn(W2, a_src2, a_dst2, H, D)
    W2f = np.concatenate([W2, av2, bv2], axis=1).astype(np.float32)     # [256,264]
    av3 = (W3 @ a_src3[0])[:, None].astype(np.float32)
    bv3 = (W3 @ a_dst3[0])[:, None].astype(np.float32)
    W3f = np.concatenate([W3, av3, bv3], axis=1).astype(np.float32)     # [256,66]

    nc = _build_program(TT, tpb, (M1, W2f, W3f, g1, b1, g2, b2, g3, b3))

    xshT = np.zeros((NC_, IN, NSHP), np.float32)
    for c in range(NC_):
        xshT[c, :, :NSH] = x[c * NSH:(c + 1) * NSH].T
    consts = {
        "i_iota": np.tile(np.arange(P, dtype=np.float32)[None, :], (P, 1)),
        "i_ident": np.eye(P, dtype=np.float32),
        "i_ones1": np.ones((1, P), np.float32),
        "i_M1": M1, "i_W2f": W2f, "i_W3f": W3f,
        "i_g1": g1[None, :], "i_b1": b1[None, :],
        "i_g2": g2[None, :], "i_b2": b2[None, :],
        "i_g3": g3[None, :], "i_b3": b3[None, :],
    }
    in_maps = []
    for c in range(NC_):
        in_maps.append({
            "i_src": src_slab[c], "i_dcol": dcol_slab[c],
            "i_xsrcT": xsrcT_slab[c], "i_xshT": xshT[c], **consts})

    res = bass_utils.run_bass_kernel_spmd(nc, in_maps,
                                          core_ids=list(range(NC_)))
    h3 = np.concatenate(
        [res.results[c]["o_h3"][:NSH] for c in range(NC_)], axis=0)

    # ---- host: graph pooling + classifier MLP (fp32 mirror of reference)
    counts = np.bincount(batch, minlength=G).astype(np.float32)
    h_sum = np.zeros((G, D), np.float32)
    np.add.at(h_sum, batch, h3)
    h_mean = h_sum / (counts[:, None] + 1.0)
    h_max = np.full((G, D), -np.inf, np.float32)
    np.maximum.at(h_max, batch, h3)
    h_max = np.maximum(h_max, 0.0)
    gfeat = np.concatenate([h_mean, h_max], axis=1)
    z = gfeat @ Wc1 + bc1
    mu = z.mean(0); var = z.var(0)
    z = (z - mu) / np.sqrt(var + EPS) * gc1 + bnc1
    z = np.maximum(z, 0.0)
    z = np.maximum(z @ Wc2 + bc2, 0.0)
    return (z @ Wc3 + bc3).astype(np.float32)



# revision 5
# speedup vs baseline: 1.5935x; 1.5935x over previous
"""BoneMeshGAT Trainium2 kernel v2: 3-layer GAT + BN/ELU on 8 NeuronCores.

Edges partitioned by dst across 8 cores (contiguous 12.5k-node dst shards).
Per layer each core gathers per-edge source rows from a node table
(h = act @ W; bf16 256 cols = 512B rows for L1/L2, f32 64 cols = 256B for
L3) with InstDMAGatherAnt (mlp gpsimd library), up to 1024 rows per
gather, from 4 balanced sub-table views (int16 index range). Attention
logits are computed on device: the src part via multiply+reduce against a
replicated a_src row; the dst part via a selection-transpose matmul
against per-block dst coefficients. Scatter-add into per-dst-block PSUM
via selection-matrix matmuls (bf16). BN statistics are accumulated with
ones-vector matmuls, AllReduced, and the next layer's table is built
per-shard and AllGathered. The program is JIT-built per input graph.
"""
import sys
sys.path.insert(0, "/opt/trn_rl_repo")
import numpy as np
import ml_dtypes

import concourse.bass as bass
import concourse.mybir as mybir
import concourse.tile as tile
from concourse import bacc, bass_utils, library_config

N, E, G = 100000, 1000000, 16
H, D, IN = 4, 64, 4
HID = H * D                 # 256
NC_ = 8
P = 128
NSH = N // NC_              # 12500
NBLK = (NSH + P - 1) // P   # 98
NSHP = NBLK * P             # 12544
NTBL = NC_ * NSHP           # 100352 padded table rows
NQ = 4
QT = NTBL // NQ             # 25088 rows per sub-table view (< 32768)
CHT = 8                     # max tiles per gather chunk (1024 idxs)
EPS = 1e-5
F32 = mybir.dt.float32
F16 = mybir.dt.float16
I32 = mybir.dt.int32
I16 = mybir.dt.int16
BF = np.float16


def _host_prep(edge_index):
    src = edge_index[0].astype(np.int64)
    dst = edge_index[1].astype(np.int64)
    order = np.argsort(dst, kind='stable')
    s_s, d_s = src[order], dst[order]
    core_bounds = np.searchsorted(d_s, np.arange(0, N + 1, NSH))

    per_core = []
    cnt = np.zeros((NC_, NBLK, NQ), np.int64)
    for c in range(NC_):
        e0, e1 = core_bounds[c], core_bounds[c + 1]
        sc = s_s[e0:e1]
        dloc = d_s[e0:e1] - c * NSH
        row = (sc // NSH) * NSHP + (sc % NSH)
        q = row // QT
        loc = (row - q * QT).astype(np.int64)
        blk = dloc // P
        col = (dloc % P).astype(np.float16)
        key = blk * NQ + q
        ordk = np.argsort(key, kind='stable')
        cnt[c] = np.bincount(key[ordk], minlength=NBLK * NQ) \
            .reshape(NBLK, NQ)
        per_core.append((loc[ordk], col[ordk], key[ordk]))

    run_tiles = -(-cnt.max(axis=0) // P)        # [NBLK, NQ] shared
    stream_tiles = run_tiles.sum(axis=0)        # [NQ]
    TT = int(run_tiles.sum())
    run_off = np.cumsum(run_tiles, axis=0) - run_tiles  # stream-local
    qbase = np.zeros(NQ + 1, np.int64)
    qbase[1:] = np.cumsum(stream_tiles)         # global tile base of stream q
    run_cum = np.cumsum(run_tiles, axis=0)      # for block lookup

    # chunk layout per stream (shared)
    chunks = []                 # (q, st0, ntile, col_ofs_16)
    chunk_of = {}               # (q, st) -> (chunk_id, slot)
    colofs = 0
    for qq in range(NQ):
        st = 0
        while st < stream_tiles[qq]:
            nt = min(CHT, int(stream_tiles[qq]) - st)
            cid = len(chunks)
            chunks.append((qq, st, nt, colofs))
            for k in range(nt):
                chunk_of[(qq, st + k)] = (cid, k)
            colofs += nt * P // 16
            st += nt
    TOTC16 = colofs

    block_tiles = []
    for j in range(NBLK):
        lst = []
        for qq in range(NQ):
            for k in range(int(run_tiles[j][qq])):
                lst.append((qq, int(run_off[j][qq]) + k))
        block_tiles.append(lst)

    idxw = np.zeros((NC_, P, TOTC16), np.int16)
    dcol_slab = np.full((NC_, P, TT), 999.0, np.float16)
    dcolT = np.full((NC_, 1, TT * P), 999.0, np.float16)
    for c in range(NC_):
        loc_s, col_s, key_s = per_core[c]
        idx_full = np.zeros((TT * P,), np.int16)
        kstart = np.searchsorted(key_s, np.arange(NBLK * NQ))
        kend = np.searchsorted(key_s, np.arange(NBLK * NQ) + 1)
        for j in range(NBLK):
            for qq in range(NQ):
                a, b = kstart[j * NQ + qq], kend[j * NQ + qq]
                n = b - a
                if n == 0:
                    continue
                tg0 = int(qbase[qq] + run_off[j][qq])   # global tile index
                s0 = tg0 * P
                idx_full[s0:s0 + n] = loc_s[a:b].astype(np.int16)
                t_idx = tg0 + (np.arange(n) // P)
                p_idx = np.arange(n) % P
                dcol_slab[c, p_idx, t_idx] = col_s[a:b]
                dcolT[c, 0, t_idx * P + p_idx] = col_s[a:b]
        for (qq, st0, nt, co) in chunks:
            g0 = int(qbase[qq] + st0) * P
            ni = nt * P
            seg = idx_full[g0:g0 + ni]
            idxw[c, :, co:co + ni // 16] = \
                np.tile(seg.reshape(ni // 16, 16).T, (8, 1))
    return (run_tiles, run_cum, qbase, TT, chunks, chunk_of, block_tiles,
            TOTC16, idxw, dcol_slab, dcolT)


def _build_program(struct):
    (run_tiles, run_cum, qbase, TT, chunks, chunk_of, block_tiles,
     TOTC16) = struct
    nc = bacc.Bacc("TRN2", target_bir_lowering=False, debug=False,
                   num_devices=NC_)

    def din(name, shape, dt=F32):
        return nc.dram_tensor(name, shape, dt, kind="ExternalInput").ap()

    i_idxw = din("i_idxw", [P, TOTC16], I16)
    i_dcol = din("i_dcol", [P, TT], F16)
    i_dcolT = din("i_dcolT", [1, TT * P], F16)
    i_tbl1 = din("i_tbl1", [NTBL, HID], F16)
    i_sdst1 = din("i_sdst1", [NSHP, H], F16)
    i_asrc1 = din("i_asrc1", [1, HID])
    i_asrc2 = din("i_asrc2", [1, HID])
    i_asrc3 = din("i_asrc3", [1, D])
    i_iota = din("i_iota", [P, P], F16)
    i_iotac = din("i_iotac", [P, 1], F16)
    i_ones1 = din("i_ones1", [1, P])
    i_W2c = din("i_W2c", [HID, HID + H], F16)   # [W2 | bv2]
    i_W3c = din("i_W3c", [HID, D + 1], F16)     # [W3 | bv3]
    i_g1 = din("i_g1", [1, HID]); i_b1 = din("i_b1", [1, HID])
    i_g2 = din("i_g2", [1, HID]); i_b2 = din("i_b2", [1, HID])
    i_g3 = din("i_g3", [1, D]);   i_b3 = din("i_b3", [1, D])
    o_h3 = nc.dram_tensor("o_h3", [NSHP, D], F32, kind="ExternalOutput").ap()

    with tile.TileContext(nc) as tc:
        with (
            tc.tile_pool(name="dram", bufs=1, space="DRAM") as dp,
            tc.tile_pool(name="const", bufs=1) as cp,
        ):
            agg1 = dp.tile([NSHP, HID], F32)
            agg2 = dp.tile([NSHP, HID], F32)
            agg3 = dp.tile([NSHP, D], F32)
            t2_own = dp.tile([NSHP, HID], F16)
            t2_full = dp.tile([NTBL, HID], F16, addr_space="Shared")
            t3_own = dp.tile([NSHP, D], F32)
            t3_full = dp.tile([NTBL, D], F32, addr_space="Shared")
            sdst2 = dp.tile([NSHP, H], F16)
            sdst3 = dp.tile([NSHP, 1], F16)
            st1_in = dp.tile([2, HID], F32)
            st1_out = dp.tile([2, HID], F32, addr_space="Shared")
            st2_in = dp.tile([2, HID], F32)
            st2_out = dp.tile([2, HID], F32, addr_space="Shared")
            st3_in = dp.tile([2, D], F32)
            st3_out = dp.tile([2, D], F32, addr_space="Shared")

            nc.gpsimd.load_library(library_config.mlp)

            def ldbf(name, iap, shape):
                f = cp.tile(shape, F32, tag=name + "_f")
                nc.sync.dma_start(f[:], iap[:])
                b = cp.tile(shape, F16, tag=name)
                nc.vector.tensor_copy(b[:], f[:])
                return b

            def ld16(name, iap, shape):
                t = cp.tile(shape, F16, tag=name)
                nc.sync.dma_start(t[:], iap[:])
                return t

            iota = ld16("iota", i_iota, [P, P])
            iotac = ld16("iotac", i_iotac, [P, 1])
            dcol = ld16("dcol", i_dcol, [P, TT])
            ones1 = cp.tile([1, P], F32, tag="ones1")
            nc.sync.dma_start(ones1[:], i_ones1[:])
            onesc = cp.tile([P, 1], F16)
            nc.vector.memset(onesc[:], 1.0)
            ident = cp.tile([P, P], F16)
            nc.vector.tensor_tensor(
                out=ident[:], in0=iotac[:].to_broadcast([P, P]),
                in1=iota[:], op=mybir.AluOpType.is_equal)

            def asrc_rep(name, iap, cw):
                row = cp.tile([1, cw], F32, tag=name + "_row")
                nc.sync.dma_start(row[:], iap[:])
                with tc.tile_pool(name="pr" + name, bufs=1,
                                  space="PSUM") as pr:
                    ps = pr.tile([P, cw], F32, tag="ps")
                    nc.tensor.matmul(out=ps[:], lhsT=ones1[:], rhs=row[:],
                                     start=True, stop=True)
                    rep = cp.tile([P, cw], F16, tag=name)
                    nc.vector.tensor_copy(rep[:], ps[:])
                return rep

            asrc1 = asrc_rep("as1", i_asrc1, HID)
            asrc2 = asrc_rep("as2", i_asrc2, HID)
            asrc3 = asrc_rep("as3", i_asrc3, D)

            def edge_phase(lay, table_ap, tdt, sdst_ap, asrc, numw, Hn,
                           agg_ap, st_in_ap):
                rhw = numw + Hn
                with (
                    tc.tile_pool(name=f"gi{lay}", bufs=12) as gi,
                    tc.tile_pool(name=f"gg{lay}", bufs=12) as gg,
                    tc.tile_pool(name=f"lw{lay}", bufs=12) as lw,
                    tc.tile_pool(name=f"sw{lay}", bufs=3) as sw,
                    tc.tile_pool(name=f"bb{lay}", bufs=4) as bb,
                    tc.tile_pool(name=f"ps{lay}", bufs=2, space="PSUM") as pp,
                    tc.tile_pool(name=f"pa{lay}", bufs=2, space="PSUM") as pa,
                    tc.tile_pool(name=f"pq{lay}", bufs=1, space="PSUM") as pq,
                ):
                    s0 = pq.tile([1, numw], F32, tag="s0")
                    s1 = pq.tile([1, numw], F32, tag="s1")
                    issued = {}
                    computed = {}
                    sdbs = {}

                    def issue(cid):
                        qq, st0, nt, co = chunks[cid]
                        ni = nt * P
                        w = gi.tile([P, CHT * P // 16], I16, tag="w")
                        nc.sync.dma_start(w[:, :ni // 16],
                                          i_idxw[:, co:co + ni // 16])
                        g = gg.tile([P, CHT * numw], tdt, tag="g")
                        lo, hi = qq * QT, (qq + 1) * QT
                        nc.gpsimd.dma_gather(
                            g[:, :nt * numw].rearrange("p (t c) -> p t c",
                                                       c=numw),
                            table_ap[lo:hi, :], w[:, :ni // 16],
                            ni, ni, numw)
                        issued[cid] = g

                    def load_sdb(j):
                        sdb = bb.tile([P, Hn], F16, tag="sdb")
                        nc.sync.dma_start(
                            sdb[:], sdst_ap[j * P:(j + 1) * P, :])
                        sdbs[j] = sdb

                    def compute(cid):
                        qq, st0, nt, co = chunks[cid]
                        ni = nt * P
                        g = issued[cid]
                        t0 = int(qbase[qq]) + st0
                        sel = lw.tile([P, CHT * P], F16, tag="sel")
                        nc.vector.tensor_tensor(
                            out=sel[:, :ni].rearrange("p (k c) -> p k c",
                                                      k=nt),
                            in0=dcol[:, t0:t0 + nt].unsqueeze(2)
                                .broadcast_to([P, nt, P]),
                            in1=iota[:].unsqueeze(1).broadcast_to([P, nt, P]),
                            op=mybir.AluOpType.is_equal)
                        drf = sw.tile([P, CHT * P], F16, tag="drf")
                        nc.sync.dma_start(
                            drf[:, :ni], i_dcolT[0:1, t0 * P:t0 * P + ni]
                            .partition_broadcast(P))
                        selT = sw.tile([P, CHT * P], F16, tag="selT")
                        nc.vector.tensor_tensor(
                            out=selT[:, :ni],
                            in0=iotac[:].to_broadcast([P, ni]),
                            in1=drf[:, :ni], op=mybir.AluOpType.is_equal)
                        prod = sw.tile([P, CHT * numw], F16, tag="prod")
                        nc.vector.tensor_tensor(
                            out=prod[:, :nt * numw]
                                .rearrange("p (k h d) -> p k h d",
                                           k=nt, h=Hn),
                            in0=g[:, :nt * numw]
                                .rearrange("p (k h d) -> p k h d",
                                           k=nt, h=Hn),
                            in1=asrc[:].rearrange("p (h d) -> p h d", h=Hn)
                                .unsqueeze(1).broadcast_to([P, nt, Hn, D]),
                            op=mybir.AluOpType.mult)
                        lgs = sw.tile([P, CHT * Hn], F32, tag="lgs")
                        nc.vector.tensor_reduce(
                            out=lgs[:, :nt * Hn]
                                .rearrange("p (k h) -> p k h", k=nt),
                            in_=prod[:, :nt * numw]
                                .rearrange("p (k h d) -> p k h d",
                                           k=nt, h=Hn),
                            axis=mybir.AxisListType.X,
                            op=mybir.AluOpType.add)
                        sdps = pp.tile([P, CHT * Hn], F32, tag="sdps")
                        for k in range(nt):
                            jj = int(np.searchsorted(run_cum[:, qq],
                                                     st0 + k, side='right'))
                            if jj not in sdbs:
                                load_sdb(jj)
                            nc.tensor.matmul(
                                out=sdps[:, k * Hn:(k + 1) * Hn],
                                lhsT=selT[:, k * P:(k + 1) * P],
                                rhs=sdbs[jj][:], start=True, stop=True)
                        lg = sw.tile([P, CHT * Hn], F32, tag="lg")
                        nc.scalar.copy(lg[:, :nt * Hn], sdps[:, :nt * Hn])
                        nc.vector.tensor_add(lg[:, :nt * Hn],
                                             lg[:, :nt * Hn],
                                             lgs[:, :nt * Hn])
                        lk = sw.tile([P, CHT * Hn], F16, tag="lk")
                        nc.vector.scalar_tensor_tensor(
                            out=lk[:, :nt * Hn], in0=lg[:, :nt * Hn],
                            scalar=0.2, in1=lg[:, :nt * Hn],
                            op0=mybir.AluOpType.mult,
                            op1=mybir.AluOpType.max)
                        ex = sw.tile([P, CHT * Hn], F16, tag="ex")
                        nc.scalar.activation(ex[:, :nt * Hn], lk[:, :nt * Hn],
                                             mybir.ActivationFunctionType.Exp)
                        rhs = lw.tile([P, CHT * rhw], F16, tag="rhs")
                        nc.vector.tensor_tensor(
                            out=rhs[:, :nt * rhw]
                                .rearrange("p (k c) -> p k c", k=nt)
                                [:, :, 0:numw]
                                .rearrange("p k (h d) -> p k h d", h=Hn),
                            in0=g[:, :nt * numw]
                                .rearrange("p (k h d) -> p k h d",
                                           k=nt, h=Hn),
                            in1=ex[:, :nt * Hn]
                                .rearrange("p (k h) -> p k h", k=nt)
                                .unsqueeze(3).broadcast_to([P, nt, Hn, D]),
                            op=mybir.AluOpType.mult)
                        nc.scalar.copy(
                            rhs[:, :nt * rhw]
                            .rearrange("p (k c) -> p k c", k=nt)
                            [:, :, numw:rhw],
                            ex[:, :nt * Hn]
                            .rearrange("p (k h) -> p k h", k=nt))
                        computed[cid] = (sel, rhs)

                    first_need = [[] for _ in range(NBLK)]
                    seen = set()
                    for j in range(NBLK):
                        for (qq, st) in block_tiles[j]:
                            cid, _ = chunk_of[(qq, st)]
                            if cid not in seen:
                                seen.add(cid)
                                first_need[j].append(cid)

                    for j in range(NBLK):
                        if j not in sdbs:
                            load_sdb(j)
                        for jj in (j, j + 1):
                            if jj < NBLK:
                                for cid in first_need[jj]:
                                    if cid not in issued:
                                        issue(cid)
                        ntile = len(block_tiles[j])
                        acc = pa.tile([P, rhw], F32, tag="acc")
                        for i, (qq, st) in enumerate(block_tiles[j]):
                            cid, slot = chunk_of[(qq, st)]
                            if cid not in computed:
                                compute(cid)
                            sel, rhs = computed[cid]
                            nc.tensor.matmul(
                                out=acc[:],
                                lhsT=sel[:, slot * P:(slot + 1) * P],
                                rhs=rhs[:, slot * rhw:(slot + 1) * rhw],
                                start=(i == 0), stop=(i == ntile - 1))
                        dn = bb.tile([P, Hn], F32, tag="dn")
                        nc.vector.tensor_scalar_add(
                            dn[:], acc[:, numw:rhw], 1e-8)
                        rec = bb.tile([P, Hn], F32, tag="rec")
                        nc.vector.reciprocal(rec[:], dn[:])
                        aggb = bb.tile([P, numw], F32, tag="aggb")
                        nc.vector.tensor_tensor(
                            out=aggb[:].rearrange("p (h d) -> p h d", h=Hn),
                            in0=acc[:, 0:numw]
                                .rearrange("p (h d) -> p h d", h=Hn),
                            in1=rec[:].unsqueeze(2).broadcast_to([P, Hn, D]),
                            op=mybir.AluOpType.mult)
                        nc.sync.dma_start(agg_ap[j * P:(j + 1) * P, :],
                                          aggb[:])
                        ab = bb.tile([P, numw], F16, tag="ab")
                        nc.scalar.copy(ab[:], aggb[:])
                        sq = bb.tile([P, numw], F16, tag="sq")
                        nc.vector.tensor_mul(sq[:], ab[:], ab[:])
                        nc.tensor.matmul(out=s0[:], lhsT=onesc[:], rhs=ab[:],
                                         start=(j == 0), stop=(j == NBLK - 1))
                        nc.tensor.matmul(out=s1[:], lhsT=onesc[:], rhs=sq[:],
                                         start=(j == 0), stop=(j == NBLK - 1))
                    c0 = bb.tile([1, numw], F32, tag="c0")
                    nc.vector.tensor_copy(c0[:], s0[:])
                    c1 = bb.tile([1, numw], F32, tag="c1")
                    nc.vector.tensor_copy(c1[:], s1[:])
                    nc.sync.dma_start(st_in_ap[0:1, :], c0[:])
                    nc.sync.dma_start(st_in_ap[1:2, :], c1[:])

            def stats_ar(st_in_ap, st_out_ap):
                nc.gpsimd.collective_compute(
                    "AllReduce", mybir.AluOpType.add,
                    ins=[st_in_ap], outs=[st_out_ap],
                    replica_groups=[list(range(NC_))])

            def bn_params(st_out_ap, CH, g_ap, b_ap, pool, tag=""):
                st0 = pool.tile([1, CH], F32, tag="stl0")
                nc.sync.dma_start(st0[:], st_out_ap[0:1, :])
                st1_ = pool.tile([1, CH], F32, tag="stl1")
                nc.sync.dma_start(st1_[:], st_out_ap[1:2, :])
                grow = pool.tile([1, CH], F32, tag="grow")
                nc.sync.dma_start(grow[:], g_ap[:])
                brow = pool.tile([1, CH], F32, tag="brow")
                nc.sync.dma_start(brow[:], b_ap[:])
                mu = pool.tile([1, CH], F32, tag="mu")
                nc.vector.tensor_scalar_mul(mu[:], st0[:], 1.0 / N)
                msq = pool.tile([1, CH], F32, tag="msq")
                nc.vector.tensor_scalar_mul(msq[:], st1_[:], 1.0 / N)
                var = pool.tile([1, CH], F32, tag="var")
                nc.vector.tensor_mul(var[:], mu[:], mu[:])
                nc.vector.tensor_sub(var[:], msq[:], var[:])
                nc.vector.tensor_scalar_add(var[:], var[:], EPS)
                sd = pool.tile([1, CH], F32, tag="sd")
                nc.scalar.activation(sd[:], var[:],
                                     mybir.ActivationFunctionType.Sqrt)
                r = pool.tile([1, CH], F32, tag="r")
                nc.vector.reciprocal(r[:], sd[:])
                arf = pool.tile([1, CH], F32, tag="arf")
                nc.vector.tensor_mul(arf[:], r[:], grow[:])

                brf = pool.tile([1, CH], F32, tag="brf")
                nc.vector.tensor_mul(brf[:], mu[:], arf[:])
                nc.vector.tensor_sub(brf[:], brow[:], brf[:])

                with tc.tile_pool(name=f"pbn{tag}", bufs=1,
                                  space="PSUM") as pb:
                    aps = pb.tile([P, CH], F32, tag="aps")
                    nc.tensor.matmul(out=aps[:], lhsT=ones1[:], rhs=arf[:],
                                     start=True, stop=True)
                    A = pool.tile([P, CH], F32, tag="Arep")
                    nc.vector.tensor_copy(A[:], aps[:])
                    bps = pb.tile([P, CH], F32, tag="bps")
                    nc.tensor.matmul(out=bps[:], lhsT=ones1[:], rhs=brf[:],
                                     start=True, stop=True)
                    B = pool.tile([P, CH], F32, tag="Brep")
                    nc.vector.tensor_copy(B[:], bps[:])
                return A, B

            def bn_elu_block(xb, A, B, CH, pool):
                z = pool.tile([P, CH], F32, tag="z")
                nc.vector.tensor_mul(z[:], xb[:], A[:])
                nc.vector.tensor_add(z[:], z[:], B[:])
                m0 = pool.tile([P, CH], F32, tag="m0")
                nc.vector.tensor_scalar_min(m0[:], z[:], 0.0)
                e = pool.tile([P, CH], F32, tag="e")
                nc.scalar.activation(e[:], m0[:],
                                     mybir.ActivationFunctionType.Exp)
                p0 = pool.tile([P, CH], F32, tag="p0")
                nc.vector.tensor_scalar_max(p0[:], z[:], 0.0)
                act = pool.tile([P, CH], F32, tag="act")
                nc.vector.tensor_add(act[:], p0[:], e[:])
                nc.vector.tensor_scalar_sub(act[:], act[:], 1.0)
                return act

            def build_pass(agg_ap, st_out_ap, g_ap, b_ap, Wc_ap, CW2,
                           t_own_ap, t_dt, sd_cols, sdst_out_ap,
                           t_full_ap, tag=""):
                CH = HID
                with (
                    tc.tile_pool(name=f"bps{tag}", bufs=3) as sp,
                    tc.tile_pool(name=f"bpc{tag}", bufs=1) as cpool,
                    tc.tile_pool(name=f"bpp{tag}", bufs=2, space="PSUM") as pp,
                    tc.tile_pool(name=f"bpt{tag}", bufs=2, space="PSUM") as pt,
                ):
                    A, B = bn_params(st_out_ap, CH, g_ap, b_ap, cpool, tag)
                    Wsb0 = cpool.tile([P, CW2], F16, tag="Wsb0")
                    nc.sync.dma_start(Wsb0[:], Wc_ap[0:P, :])
                    Wsb1 = cpool.tile([P, CW2], F16, tag="Wsb1")
                    nc.sync.dma_start(Wsb1[:], Wc_ap[P:2 * P, :])
                    Wsb = [Wsb0, Wsb1]
                    nw = CW2 - sd_cols
                    for j in range(NBLK):
                        xb = sp.tile([P, CH], F32, tag="xb")
                        nc.sync.dma_start(xb[:], agg_ap[j * P:(j + 1) * P, :])
                        act = bn_elu_block(xb, A, B, CH, sp)
                        actb = sp.tile([P, CH], F16, tag="actb")
                        nc.scalar.copy(actb[:], act[:])
                        tp = pt.tile([P, CW2], F32, tag="tp")
                        for k in range(2):
                            trp = pp.tile([P, P], F16, tag="trp")
                            nc.tensor.transpose(
                                out=trp[:], in_=actb[:, k * P:(k + 1) * P],
                                identity=ident[:])
                            atk = sp.tile([P, P], F16, tag="atk")
                            nc.vector.tensor_copy(atk[:], trp[:])
                            nc.tensor.matmul(out=tp[:], lhsT=atk[:],
                                             rhs=Wsb[k][:],
                                             start=(k == 0), stop=(k == 1))
                        tt = sp.tile([P, nw], t_dt, tag="tt")
                        nc.scalar.copy(tt[:], tp[:, 0:nw])
                        nc.sync.dma_start(t_own_ap[j * P:(j + 1) * P, :],
                                          tt[:])
                        sdt = sp.tile([P, sd_cols], F16, tag="sdt")
                        nc.vector.tensor_copy(sdt[:], tp[:, nw:CW2])
                        nc.sync.dma_start(sdst_out_ap[j * P:(j + 1) * P, :],
                                          sdt[:])
                nc.gpsimd.collective_compute(
                    "AllGather", mybir.AluOpType.bypass,
                    ins=[t_own_ap], outs=[t_full_ap],
                    replica_groups=[list(range(NC_))])

            # ================= pipeline =================
            edge_phase(1, i_tbl1, F16, i_sdst1, asrc1, HID, H,
                       agg1[:], st1_in[:])
            stats_ar(st1_in[:], st1_out[:])
            build_pass(agg1[:], st1_out[:], i_g1[:], i_b1[:], i_W2c[:],
                       HID + H, t2_own[:], F16, H, sdst2[:], t2_full[:],
                       tag="1")
            edge_phase(2, t2_full[:], F16, sdst2[:], asrc2, HID, H,
                       agg2[:], st2_in[:])
            stats_ar(st2_in[:], st2_out[:])
            build_pass(agg2[:], st2_out[:], i_g2[:], i_b2[:], i_W3c[:],
                       D + 1, t3_own[:], F32, 1, sdst3[:], t3_full[:],
                       tag="2")
            edge_phase(3, t3_full[:], F32, sdst3[:], asrc3, D, 1,
                       agg3[:], st3_in[:])
            stats_ar(st3_in[:], st3_out[:])
            with (
                tc.tile_pool(name="f3", bufs=3) as sp,
                tc.tile_pool(name="f3c", bufs=1) as cpool,
            ):
                A3, B3 = bn_params(st3_out[:], D, i_g3[:], i_b3[:], cpool,
                                   "3")
                for j in range(NBLK):
                    xb = sp.tile([P, D], F32, tag="xb")
                    nc.sync.dma_start(xb[:], agg3[j * P:(j + 1) * P, :])
                    act = bn_elu_block(xb, A3, B3, D, sp)
                    nc.sync.dma_start(o_h3[j * P:(j + 1) * P, :], act[:])
    nc.compile()
    return nc


def kernel(**inputs):
    x = np.asarray(inputs["x"], np.float32)
    edge_index = np.asarray(inputs["edge_index"], np.int32)
    batch = np.asarray(inputs["batch"], np.int64)
    W1 = np.asarray(inputs["W1"], np.float32)
    a_src1 = np.asarray(inputs["a_src1"], np.float32)
    a_dst1 = np.asarray(inputs["a_dst1"], np.float32)
    g1 = np.asarray(inputs["g1"], np.float32); b1 = np.asarray(inputs["b1"], np.float32)
    W2 = np.asarray(inputs["W2"], np.float32)
    a_src2 = np.asarray(inputs["a_src2"], np.float32)
    a_dst2 = np.asarray(inputs["a_dst2"], np.float32)
    g2 = np.asarray(inputs["g2"], np.float32); b2 = np.asarray(inputs["b2"], np.float32)
    W3 = np.asarray(inputs["W3"], np.float32)
    a_src3 = np.asarray(inputs["a_src3"], np.float32)
    a_dst3 = np.asarray(inputs["a_dst3"], np.float32)
    g3 = np.asarray(inputs["g3"], np.float32); b3 = np.asarray(inputs["b3"], np.float32)
    Wc1 = np.asarray(inputs["Wc1"], np.float32); bc1 = np.asarray(inputs["bc1"], np.float32)
    gc1 = np.asarray(inputs["gc1"], np.float32); bnc1 = np.asarray(inputs["bnc1"], np.float32)
    Wc2 = np.asarray(inputs["Wc2"], np.float32); bc2 = np.asarray(inputs["bc2"], np.float32)
    Wc3 = np.asarray(inputs["Wc3"], np.float32); bc3 = np.asarray(inputs["bc3"], np.float32)

    (run_tiles, run_cum, qbase, TT, chunks, chunk_of, block_tiles,
     TOTC16, idxw, dcol_slab, dcolT) = _host_prep(edge_index)

    h1 = x @ W1                               # [N, 256]
    tbl1 = np.zeros((NTBL, HID), BF)
    for c in range(NC_):
        tbl1[c * NSHP:c * NSHP + NSH] = h1[c * NSH:(c + 1) * NSH].astype(BF)
    sdst1 = np.zeros((NC_, NSHP, H), np.float16)
    ad1 = np.einsum('nhd,hd->nh', h1.reshape(N, H, D),
                    a_dst1).astype(np.float16)
    for c in range(NC_):
        sdst1[c, :NSH] = ad1[c * NSH:(c + 1) * NSH]
    bv2 = np.einsum('ihd,hd->ih', W2.reshape(HID, H, D), a_dst2)
    W2c = np.concatenate([W2, bv2], axis=1).astype(BF)        # [256, 260]
    bv3 = (W3 @ a_dst3[0])[:, None]
    W3c = np.concatenate([W3, bv3], axis=1).astype(BF)        # [256, 65]

    nc = _build_program((run_tiles, run_cum, qbase, TT, chunks,
                         chunk_of, block_tiles, TOTC16))

    consts = {
        "i_iota": np.tile(np.arange(P, dtype=np.float16)[None, :], (P, 1)),
        "i_iotac": np.arange(P, dtype=np.float16)[:, None],
        "i_ones1": np.ones((1, P), np.float32),
        "i_tbl1": tbl1,
        "i_asrc1": a_src1.reshape(1, HID).astype(np.float32),
        "i_asrc2": a_src2.reshape(1, HID).astype(np.float32),
        "i_asrc3": a_src3.reshape(1, D).astype(np.float32),
        "i_W2c": W2c, "i_W3c": W3c,
        "i_g1": g1[None, :], "i_b1": b1[None, :],
        "i_g2": g2[None, :], "i_b2": b2[None, :],
        "i_g3": g3[None, :], "i_b3": b3[None, :],
    }
    in_maps = []
    for c in range(NC_):
        in_maps.append({
            "i_idxw": idxw[c], "i_dcol": dcol_slab[c],
            "i_dcolT": dcolT[c], "i_sdst1": sdst1[c], **consts})

    res = bass_utils.run_bass_kernel_spmd(nc, in_maps,
                                          core_ids=list(range(NC_)))
    h3 = np.concatenate(
        [res.results[c]["o_h3"][:NSH] for c in range(NC_)], axis=0)

    counts = np.bincount(batch, minlength=G).astype(np.float32)
    h_sum = np.zeros((G, D), np.float32)
    np.add.at(h_sum, batch, h3)
    h_mean = h_sum / (counts[:, None] + 1.0)
    h_max = np.full((G, D), -np.inf, np.float32)
    np.maximum.at(h_max, batch, h3)
    h_max = np.maximum(h_max, 0.0)
    gfeat = np.concatenate([h_mean, h_max], axis=1)
    z = gfeat @ Wc1 + bc1
    mu = z.mean(0); var = z.var(0)
    z = (z - mu) / np.sqrt(var + EPS) * gc1 + bnc1
    z = np.maximum(z, 0.0)
    z = np.maximum(z @ Wc2 + bc2, 0.0)
    return (z @ Wc3 + bc3).astype(np.float32)
